# revision 4
# baseline (speedup 1.0000x reference)
"""Trainium2 Bass kernel for nn_BaselineAttn (LoRA QKV + ALiBi causal attention).

Sharding: 8 cores SPMD, no collectives. Core c = (b, g): batch b = c // 4,
head group g = c % 4 handling heads [g, 4+g, 8+g, 12+g].

Host prep: LoRA folded into weights (W' = W + 2 A@B); x and weights
pre-transposed/sliced per core; partial outputs summed on host.

Device design (fp16 operands, fp32 PSUM):
  - feature-major x^T on chip -> q^T, k^T feature-major and v token-major
    from the same x^T; zero on-chip transposes.
  - attention in the S^T (key-major) orientation:
      S^T tile = k^T-tile.T @ q^T-chunk
      P^T = exp(S^T/8 + bias_k), bias_k = -slope_h*k per-PARTITION: ALiBi +
        softmax shift fused into one ScalarE activation.
      causal: only the 128-wide diagonal block of each diagonal-band tile is
        partially masked (cols >= j0+128 are fully valid), so one shared
        [128,128] triangular mask multiply per diagonal tile; dead tiles
        skipped; per-tile active q-range sliced.
      O^T += vext.T @ P^T where vext = [ones*64 | v]: OT matmul free time
        doesn't depend on out-partitions, so the ones block replicates the
        softmax denominator onto partitions 0..63 (base partition 0 so the
        DVE fast-reciprocal can read it straight out of PSUM).
      normalize: DVE fast-reciprocal on PSUM rows 0:64, then one DVE mul.
      out-partial = O^T_norm.T @ Wp'^T-slice, written f16 (host sums f32).
  - ALiBi gives key k weight exp(-slope_h*k); p is stored f16 whose
    subnormal floor is ~e^-17, so tiles beyond slope_h*128*kt > ~16 are
    exactly zero anyway and are skipped: SNKT = [1, 2, 8, 16].

Perf notes (vs the first working version):
  - inputs arrive via 9 large contiguous DMAs (each dma_start costs ~600ns
    of HWDGE sequencer time, so fewer+bigger is faster to issue) with the
    first-matmul working set (x chunk 0 + wq) triggered first, split across
    the two HWDGE rings (sync + scalar).
  - a chain of throwaway warm-up matmuls runs while the first DMAs land so
    the PE HAM clock-gate is already at 8/8 when real work starts, plus a
    dummy exp to hoist the ~2.7us ACT table load into the DMA window.
  - S^T matmuls for the two slots of a pair (contraction rows 0:64 / 64:128)
    are emitted back-to-back so the PE can run them concurrently in
    different row-groups.
  - PSUM->SBUF copy work split ACT/DVE by measured engine occupancy.
"""

import math

import numpy as np

E = 1024
H = 16
DH = 64
T = 2048
BATCH = 2
LORA_S = 2.0
NKT = T // 128          # 16 key tiles of 128
SNKT = [1, 2, 8, 16]    # per-slot key-tile caps (max over cores per slot)
NQC = 4                 # q chunks of 512
N_WARMUP = 78           # warm-up matmuls (N=256) covering the input-DMA window

_NC_CACHE = {}


def _slopes():
    start = 2 ** (-2 ** (-(math.log2(H) - 3)))
    return np.array([start * start**i for i in range(H)], dtype=np.float64)


def _smin(tt):
    """Lowest slot that still needs key-tile tt."""
    for s in range(4):
        if tt < SNKT[s]:
            return s
    return 4


def _build_nc():
    """Build the single SPMD Bass program (shared by all 8 cores)."""
    if "nc" in _NC_CACHE:
        return _NC_CACHE["nc"]

    from concourse.bacc import Bacc
    import concourse.tile as tile
    from concourse import mybir

    f16 = mybir.dt.float16
    f32 = mybir.dt.float32
    EXP = mybir.ActivationFunctionType.Exp

    nc = Bacc()

    xc_d = [nc.dram_tensor(f"xc{c}", [128, 8, 512], f16, kind="ExternalInput")
            for c in range(NQC)]
    wq_d = nc.dram_tensor("wqT", [128, 8, 256], f16, kind="ExternalInput")
    wkv_d = nc.dram_tensor("wkvT", [128, 8, 512], f16, kind="ExternalInput")
    wp_d = nc.dram_tensor("wpT", [128, 2, 1024], f16, kind="ExternalInput")
    bias_d = nc.dram_tensor("expbias", [128, 64], f32, kind="ExternalInput")
    mask_d = nc.dram_tensor("masks", [128, 128], f16, kind="ExternalInput")
    out_d = nc.dram_tensor("outp", [T, E], f16, kind="ExternalOutput")
    scr_d = nc.dram_tensor("scratch", [128, 8], f16, kind="ExternalOutput")

    with tile.TileContext(nc) as tc:
        with (
            tc.tile_pool(name="persist", bufs=1) as pp,
            tc.tile_pool(name="ptpool", bufs=10) as ptp,
            tc.tile_pool(name="onorm", bufs=4) as onp,
            tc.tile_pool(name="rpool", bufs=4) as rp,
            tc.tile_pool(name="outsb", bufs=6) as osp,
        ):
            # ---- PE warm-up source, independent of DMA ----
            wm_sb = pp.tile([128, 256], f16, name="wm_sb")
            nc.gpsimd.memset(wm_sb, 0.5)
            wexp = pp.tile([128, 8], f16, name="wexp")
            scr_sb = pp.tile([128, 8], f16, name="scr_sb")

            # ---- input loads: few, large, contiguous; first-MM set first --
            xT = [pp.tile([128, 8, 512], f16, name=f"xT{c}") for c in range(NQC)]
            wq_sb = pp.tile([128, 8, 256], f16, name="wq_sb")
            wkv_sb = pp.tile([128, 8, 512], f16, name="wkv_sb")
            nc.sync.dma_start(out=xT[0], in_=xc_d[0][:, :, :])
            nc.scalar.dma_start(out=wq_sb, in_=wq_d[:, :, :])
            nc.sync.dma_start(out=wkv_sb, in_=wkv_d[:, :, :])
            bias_sb = pp.tile([128, 64], f32, name="bias")
            nc.scalar.dma_start(out=bias_sb, in_=bias_d[:, :])
            mask_sb = pp.tile([128, 128], f16, name="mask")
            nc.scalar.dma_start(out=mask_sb, in_=mask_d[:, :])
            for c in range(1, NQC):
                nc.sync.dma_start(out=xT[c], in_=xc_d[c][:, :, :])
            wp_sb = pp.tile([128, 2, 1024], f16, name="wp_sb")
            nc.scalar.dma_start(out=wp_sb, in_=wp_d[:, :, :])
            # dummy exp hoists the ~1.3us ACT table load into the DMA window
            # (after the scalar-ring DMA triggers so it doesn't delay them).
            nc.scalar.activation(out=wexp, in_=wm_sb[:, 0:8], func=EXP,
                                 bias=0.0, scale=0.125)

            # warm-up matmuls keep the PE busy (and its clock un-throttled)
            # while the first input DMAs land; results are dead.
            with tc.tile_pool(name="wups", bufs=1, space="PSUM") as wups:
                wacc = wups.tile([128, 512], f32, name="wacc")
                for _ in range(N_WARMUP):
                    nc.tensor.matmul(wacc[:, 0:256], wm_sb[:, 0:128], wm_sb,
                                     start=True, stop=True)
                # tiny live sink so the chain can't be dead-code-eliminated
                nc.vector.tensor_copy(out=scr_sb, in_=wacc[:, 0:8])
                nc.sync.dma_start(out=scr_d[:, :], in_=scr_sb)

            # vext[tt]: [128 keys, slot, 128]: cols 0:64 = ones (denominator
            # replicas at base partition 0), cols 64:128 = v.
            vext = []
            for tt in range(NKT):
                v_t = pp.tile([128, 4, 128], f16, name=f"vext{tt}")
                nc.gpsimd.memset(v_t[:, :, 0:64], 1.0)
                vext.append(v_t)
            # q^T / k^T: per (p-tile, chunk) tiles [128, 512].
            # kT p-tile 0 (slots 0,1) only needs k < 256: chunk 0 only.
            qT = [[pp.tile([128, 512], f16, name=f"qT{p}_{ncu}") for ncu in range(NQC)]
                  for p in range(2)]
            kT = [[pp.tile([128, 512], f16, name=f"kT{p}_{ncu}")
                   if (p == 1 or ncu < 1) else None for ncu in range(NQC)]
                  for p in range(2)]

            ncopy = 0  # round-robin Act/DVE for qk PSUM->SBUF copies

            # ---- phase 1: QKV projections ----
            with tc.tile_pool(name="qkps", bufs=3, space="PSUM") as qkps, \
                 tc.tile_pool(name="vps", bufs=3, space="PSUM") as vps:
                # chunk-major emission: q,k,v for chunk ncu before chunk ncu+1,
                # so attention for q-chunk 0 can start 4x earlier.
                with nc.named_scope("qkv_proj"):
                    for ncu in range(NQC):
                        for which, dst in (("q", qT), ("k", kT)):
                            for mt in range(2):
                                if dst[mt][ncu] is None:
                                    continue
                                # kT[0][0]: only k in [0, 256) used -> N=256
                                nw = 256 if (which == "k" and mt == 0) else 512
                                acc = qkps.tile([128, 512], f32, tag="qkacc",
                                                name=f"qkacc{which}_{mt}_{ncu}")
                                for kt in range(8):
                                    w_sb = (wq_sb if which == "q" else wkv_sb)
                                    nc.tensor.matmul(
                                        acc[:, 0:nw],
                                        w_sb[:, kt, mt * 128:(mt + 1) * 128],
                                        xT[ncu][:, kt, 0:nw],
                                        start=(kt == 0), stop=(kt == 7),
                                    )
                                ncopy += 1
                                if ncopy % 2:
                                    nc.scalar.copy(out=dst[mt][ncu][:, 0:nw],
                                                   in_=acc[:, 0:nw])
                                else:
                                    nc.vector.tensor_copy(out=dst[mt][ncu][:, 0:nw],
                                                          in_=acc[:, 0:nw])
                        for tt in range(4 * ncu, 4 * ncu + 4):
                            s0 = _smin(tt)
                            if s0 >= 4:
                                continue
                            nw = (4 - s0) * 64
                            acc = vps.tile([128, 256], f32, tag="vacc", name=f"vacc{tt}")
                            for kt in range(8):
                                nc.tensor.matmul(
                                    acc[:, 0:nw],
                                    xT[ncu][:, kt, (tt % 4) * 128:(tt % 4 + 1) * 128],
                                    wkv_sb[:, kt, 256 + s0 * 64:512],
                                    start=(kt == 0), stop=(kt == 7),
                                )
                            nc.scalar.copy(
                                out=vext[tt][:, s0:4, 64:128],
                                in_=acc[:, 0:nw].rearrange("p (s d) -> p s d", d=64))

            # ---- phase 2: attention + output projection, per q-chunk ----
            # PSUM: stps 4 banks + shared ot/pacc pool 4 banks = 8.
            with tc.tile_pool(name="stps", bufs=4, space="PSUM") as stps, \
                 tc.tile_pool(name="spool", bufs=4, space="PSUM") as spool:
                nosb = 0
                for qc in range(NQC):
                    on_tiles = [onp.tile([128, 512], f16, tag="on", name=f"on_{qc}_{p}")
                                for p in range(2)]
                    # big pair (slots 2,3) first: its chains gate the proj
                    for pair in (1, 0):
                        sA, sB = 2 * pair, 2 * pair + 1
                        nktA = min(SNKT[sA], 4 * qc + 4)
                        nktB = min(SNKT[sB], 4 * qc + 4)
                        pt_i = pair
                        ot = {}
                        for s, nkt in ((sA, nktA), (sB, nktB)):
                            ot[s] = spool.tile([128, 512], f32, tag="ot",
                                               name=f"ot_{qc}_{s}")
                        with nc.named_scope(f"attn_q{qc}_p{pair}"):
                            for kt in range(max(nktA, nktB)):
                                j0 = (kt - 4 * qc) * 128 if kt >= 4 * qc else 0
                                todo = [s for s, nkt in ((sA, nktA), (sB, nktB))
                                        if kt < nkt]
                                sts = {}
                                # S^T matmuls for both slots adjacent: they hit
                                # disjoint PE row-groups (rows 0:64 / 64:128)
                                # and can run concurrently.
                                for s in todo:
                                    r0 = 64 * (s % 2)
                                    st = stps.tile([128, 512], f32, tag="st",
                                                   name=f"st_{qc}_{s}_{kt}")
                                    sts[s] = st
                                    nc.tensor.matmul(
                                        st[:, j0:512],
                                        kT[pt_i][kt // 4][r0:r0 + 64,
                                                          (kt % 4) * 128:(kt % 4 + 1) * 128],
                                        qT[pt_i][qc][r0:r0 + 64, j0:512],
                                        start=True, stop=True,
                                    )
                                pts = {}
                                for s in todo:
                                    p_t = ptp.tile([128, 512], f16, tag="pt",
                                                   name=f"pt_{qc}_{s}_{kt}")
                                    pts[s] = p_t
                                    nc.scalar.activation(
                                        out=p_t[:, j0:512], in_=sts[s][:, j0:512],
                                        func=EXP,
                                        bias=bias_sb[:, s * 16 + kt:s * 16 + kt + 1],
                                        scale=0.125,
                                    )
                                    if kt >= 4 * qc:
                                        # only the 128-wide diagonal block needs
                                        # masking; cols >= j0+128 are all valid.
                                        meng = nc.vector if s % 2 else nc.gpsimd
                                        meng.tensor_mul(
                                            out=p_t[:, j0:j0 + 128],
                                            in0=p_t[:, j0:j0 + 128],
                                            in1=mask_sb,
                                        )
                                for s in todo:
                                    nkt = nktA if s == sA else nktB
                                    nc.tensor.matmul(
                                        ot[s][:, j0:512],
                                        vext[kt][:, s, :],
                                        pts[s][:, j0:512],
                                        start=(kt == 0), stop=(kt == nkt - 1),
                                    )
                            # rows 0:64 of ot hold the softmax denominator
                            # replicated (base partition 0): fast-reciprocal
                            # straight from PSUM, then one DVE mul.
                            for s in (sA, sB):
                                r0 = 64 * (s % 2)
                                rec = rp.tile([64, 512], f32, tag="rec",
                                              name=f"rec_{qc}_{s}")
                                nc.vector.reciprocal_approx_fast(
                                    out=rec, in_=ot[s][0:64, :])
                                nc.vector.tensor_mul(
                                    out=on_tiles[pair][r0:r0 + 64, :],
                                    in0=ot[s][64:128, :],
                                    in1=rec,
                                )
                    with nc.named_scope(f"proj_q{qc}"):
                        for tloc in range(4):
                            tt = qc * 4 + tloc
                            osb = osp.tile([128, 1024], f16, tag="osb",
                                           name=f"osb_{tt}")
                            for ech in range(2):
                                pacc = spool.tile([128, 512], f32, tag="ot",
                                                  name=f"pacc_{tt}_{ech}")
                                for pt_i in (0, 1):
                                    nc.tensor.matmul(
                                        pacc,
                                        on_tiles[pt_i][:, tloc * 128:(tloc + 1) * 128],
                                        wp_sb[:, pt_i, ech * 512:(ech + 1) * 512],
                                        start=(pt_i == 0), stop=(pt_i == 1),
                                    )
                                nosb += 1
                                if nosb % 8 == 0:
                                    nc.scalar.copy(
                                        out=osb[:, ech * 512:(ech + 1) * 512],
                                        in_=pacc)
                                else:
                                    nc.vector.tensor_copy(
                                        out=osb[:, ech * 512:(ech + 1) * 512],
                                        in_=pacc)
                            nc.sync.dma_start(
                                out=out_d[tt * 128:(tt + 1) * 128, :], in_=osb)

    nc.finalize()
    _NC_CACHE["nc"] = nc
    return nc


def _prep_core_inputs(x, Wq, Aq, Bq, Wk, Ak, Bk, Wv, Av, Bv, Wp):
    """Host-side prep: LoRA fold, transposes, per-core slices."""
    slopes = _slopes()
    wq_m = Wq.astype(np.float64) + LORA_S * (Aq.astype(np.float64) @ Bq.astype(np.float64))
    wk_m = Wk.astype(np.float64) + LORA_S * (Ak.astype(np.float64) @ Bk.astype(np.float64))
    wv_m = Wv.astype(np.float64) + LORA_S * (Av.astype(np.float64) @ Bv.astype(np.float64))

    # shared [128,128] triangular mask: within a diagonal 128-block,
    # key-in-tile p is valid for local col j iff p <= j.
    p_i = np.arange(128)[:, None]
    j_i = np.arange(128)[None, :]
    masks = np.ascontiguousarray((p_i <= j_i).astype(np.float16))

    in_maps = []
    for c in range(8):
        b, g = divmod(c, 4)
        heads = [g, 4 + g, 8 + g, 12 + g]
        rows = np.concatenate([np.arange(h * DH, (h + 1) * DH) for h in heads])
        xT = x[b].T.astype(np.float16)          # [E, T]
        wqT = wq_m[rows, :].T.astype(np.float16)         # [E, 256]
        wkvT = np.concatenate(
            [wk_m[rows, :].T, wv_m[rows, :].T], axis=1).astype(np.float16)  # [E,512]
        wpT = Wp[:, rows].T.astype(np.float16)           # [256, E]
        bias = np.zeros((128, 64), dtype=np.float32)
        for s, h in enumerate(heads):
            for kt in range(16):
                bias[:, s * 16 + kt] = -slopes[h] * (kt * 128 + np.arange(128))
        im = {
            "wqT": np.ascontiguousarray(
                wqT.reshape(8, 128, 256).transpose(1, 0, 2)),
            "wkvT": np.ascontiguousarray(
                wkvT.reshape(8, 128, 512).transpose(1, 0, 2)),
            "wpT": np.ascontiguousarray(
                wpT.reshape(2, 128, 1024).transpose(1, 0, 2)),
            "expbias": bias, "masks": masks,
        }
        for cch in range(NQC):
            im[f"xc{cch}"] = np.ascontiguousarray(
                xT[:, cch * 512:(cch + 1) * 512]
                .reshape(8, 128, 512).transpose(1, 0, 2))
        in_maps.append(im)
    return in_maps


def _run(in_maps, trace=False, **kw):
    from concourse.bass_utils import run_bass_kernel_spmd
    nc = _build_nc()
    return run_bass_kernel_spmd(nc, in_maps, core_ids=list(range(8)), trace=trace, **kw)


def kernel(x, Wq, Aq, Bq, Wk, Ak, Bk, Wv, Av, Bv, Wp):
    in_maps = _prep_core_inputs(x, Wq, Aq, Bq, Wk, Ak, Bk, Wv, Av, Bv, Wp)
    res = _run(in_maps)
    out = np.zeros((BATCH, T, E), dtype=np.float32)
    for c in range(8):
        out[c // 4] += res.results[c]["outp"].astype(np.float32)
    return out


# revision 5
# speedup vs baseline: 1.0932x; 1.0932x over previous
"""Trainium2 Bass kernel for nn_BaselineAttn (LoRA QKV + ALiBi causal attention).

Sharding: 8 cores SPMD, no collectives. Core c = (b, g): batch b = c // 4,
head group g = c % 4 handling heads [g, 4+g, 8+g, 12+g].

Host prep: LoRA folded into weights (W' = W + 2 A@B); x and weights
pre-transposed/sliced per core; partial outputs summed on host.

Device design (fp16 operands, fp32 PSUM):
  - feature-major x^T on chip -> q^T, k^T feature-major and v token-major
    from the same x^T; zero on-chip transposes.
  - attention in the S^T (key-major) orientation, with the ALiBi bias
    folded into the S matmul itself: k^T tiles carry a 65th contraction row
    holding -8*slope_h*k and q^T tiles carry a matching ones row, so
    S^T = k_ext^T.T @ q_ext^T already includes the softmax-shifted ALiBi
    term and the ScalarE exp needs no per-partition bias. That lets one
    activation cover BOTH slots of a pair (their S tiles live in one
    2-bank PSUM tile), halving the exp op count.
      P^T = exp(S^T/8), causal: only the 128-wide diagonal block of each
      diagonal-band tile is partially masked -> one shared [128,128]
      triangular mask multiply (GpSimd) per diagonal tile; the OT matmul of
      a diagonal tile is split into mask-free columns (issued right after
      exp) + the masked 128-block, hiding the mask latency.
      O^T += vext.T @ P^T where vext = [ones*64 | v]: the ones block
      replicates the softmax denominator onto partitions 0..63 so the DVE
      fast-reciprocal reads it straight from PSUM (base partition 0);
      normalize is recip + one DVE mul.
      out-partial = O^T_norm.T @ Wp'^T-slice, written f16 (host sums f32).
  - ALiBi gives key k weight exp(-slope_h*k); p is stored f16 whose
    subnormal floor is ~e^-17, so tiles beyond slope_h*128*kt > ~16 are
    exactly zero anyway and are skipped: SNKT = [1, 2, 8, 16].
  - inputs arrive via a few large contiguous DMAs (each dma_start costs
    ~600ns of HWDGE sequencer issue time), first-matmul working set first,
    split across the sync/scalar HWDGE rings; a short PE warm-up chain and
    a dummy exp (ACT table load) run inside the DMA window.
"""

import math

import numpy as np

E = 1024
H = 16
DH = 64
T = 2048
BATCH = 2
LORA_S = 2.0
NKT = T // 128          # 16 key tiles of 128
SNKT = [1, 2, 8, 16]    # per-slot key-tile caps (max over cores per slot)
NQC = 4                 # q chunks of 512
N_WARMUP = 16           # warm-up matmuls (N=256) to pre-warm the PE clock

_NC_CACHE = {}


def _slopes():
    start = 2 ** (-2 ** (-(math.log2(H) - 3)))
    return np.array([start * start**i for i in range(H)], dtype=np.float64)


def _smin(tt):
    """Lowest slot that still needs key-tile tt."""
    for s in range(4):
        if tt < SNKT[s]:
            return s
    return 4


def _build_nc():
    """Build the single SPMD Bass program (shared by all 8 cores)."""
    if "nc" in _NC_CACHE:
        return _NC_CACHE["nc"]

    from concourse.bacc import Bacc
    import concourse.tile as tile
    from concourse import mybir

    f16 = mybir.dt.float16
    f32 = mybir.dt.float32
    EXP = mybir.ActivationFunctionType.Exp

    nc = Bacc()

    xc_d = [nc.dram_tensor(f"xc{c}", [128, 8, 512], f16, kind="ExternalInput")
            for c in range(NQC)]
    wq_d = nc.dram_tensor("wqT", [128, 8, 256], f16, kind="ExternalInput")
    wkv_d = nc.dram_tensor("wkvT", [128, 8, 512], f16, kind="ExternalInput")
    wp_d = nc.dram_tensor("wpT", [128, 2, 1024], f16, kind="ExternalInput")
    kbias_d = nc.dram_tensor("kbias", [1, 4, 2048], f16, kind="ExternalInput")
    qones_d = nc.dram_tensor("qones", [1, 4, 2048], f16, kind="ExternalInput")
    mask_d = nc.dram_tensor("masks", [128, 128], f16, kind="ExternalInput")
    out_d = nc.dram_tensor("outp", [T, E], f16, kind="ExternalOutput")
    scr_d = nc.dram_tensor("scratch", [128, 8], f16, kind="ExternalOutput")

    with tile.TileContext(nc) as tc:
        with (
            tc.tile_pool(name="persist", bufs=1) as pp,
            tc.tile_pool(name="ptpool", bufs=8) as ptp,
            tc.tile_pool(name="onorm", bufs=4) as onp,
            tc.tile_pool(name="rpool", bufs=4) as rp,
            tc.tile_pool(name="outsb", bufs=6) as osp,
        ):
            # ---- PE warm-up source, independent of DMA ----
            wm_sb = pp.tile([128, 256], f16, name="wm_sb")
            nc.gpsimd.memset(wm_sb, 0.5)
            wexp = pp.tile([128, 8], f16, name="wexp")
            scr_sb = pp.tile([128, 8], f16, name="scr_sb")

            # ---- input loads: few, large, contiguous; first-MM set first --
            xT = [pp.tile([128, 8, 512], f16, name=f"xT{c}") for c in range(NQC)]
            wq_sb = pp.tile([128, 8, 256], f16, name="wq_sb")
            wkv_sb = pp.tile([128, 8, 512], f16, name="wkv_sb")
            # q_ext/k_ext: [65, slot, keys]: rows 0:64 features, row 64 =
            # ones (q) / -8*slope*k ALiBi bias row (k).
            qts = pp.tile([65, 4, 2048], f16, name="qts")
            kts = pp.tile([65, 4, 2048], f16, name="kts")

            nc.sync.dma_start(out=xT[0], in_=xc_d[0][:, :, :])
            nc.scalar.dma_start(out=wq_sb, in_=wq_d[:, :, :])
            nc.sync.dma_start(out=wkv_sb, in_=wkv_d[:, :, :])
            nc.scalar.dma_start(out=kts[64:65, :, :], in_=kbias_d[:, :, :])
            nc.scalar.dma_start(out=qts[64:65, :, :], in_=qones_d[:, :, :])
            mask_sb = pp.tile([128, 128], f16, name="mask")
            nc.scalar.dma_start(out=mask_sb, in_=mask_d[:, :])
            for c in range(1, NQC):
                nc.sync.dma_start(out=xT[c], in_=xc_d[c][:, :, :])
            wp_sb = pp.tile([128, 2, 1024], f16, name="wp_sb")
            nc.scalar.dma_start(out=wp_sb, in_=wp_d[:, :, :])
            # dummy exp hoists the ~1.3us ACT table load into the DMA window
            # (after the scalar-ring DMA triggers so it doesn't delay them).
            nc.scalar.activation(out=wexp, in_=wm_sb[:, 0:8], func=EXP,
                                 bias=0.0, scale=0.125)

            # warm-up matmuls cover the PE-idle DMA window so the HAM clock
            # gate is released before real work; results are dead.
            with tc.tile_pool(name="wups", bufs=1, space="PSUM") as wups:
                wacc = wups.tile([128, 512], f32, name="wacc")
                for _ in range(N_WARMUP):
                    nc.tensor.matmul(wacc[:, 0:256], wm_sb[:, 0:128], wm_sb,
                                     start=True, stop=True)
                # tiny live sink so the chain can't be dead-code-eliminated
                nc.vector.tensor_copy(out=scr_sb, in_=wacc[:, 0:8])
                nc.sync.dma_start(out=scr_d[:, :], in_=scr_sb)

            # vext[tt]: [128 keys, slot, 128]: cols 0:64 = ones (denominator
            # replicas at base partition 0), cols 64:128 = v.
            vext = []
            for tt in range(NKT):
                v_t = pp.tile([128, 4, 128], f16, name=f"vext{tt}")
                nc.gpsimd.memset(v_t[:, :, 0:64], 1.0)
                vext.append(v_t)

            ncopy = 0  # round-robin Act/DVE for qk PSUM->SBUF copies

            # ---- phase 1: QKV projections ----
            with tc.tile_pool(name="qkps", bufs=3, space="PSUM") as qkps, \
                 tc.tile_pool(name="vps", bufs=3, space="PSUM") as vps:
                # chunk-major emission: q,k,v for chunk ncu before chunk ncu+1,
                # so attention for q-chunk 0 can start 4x earlier.
                with nc.named_scope("qkv_proj"):
                    for ncu in range(NQC):
                        for which, dst in (("q", qts), ("k", kts)):
                            for mt in range(2):
                                if which == "k" and mt == 0 and ncu >= 1:
                                    continue  # slots 0,1 need keys < 256 only
                                nw = 256 if (which == "k" and mt == 0) else 512
                                acc = qkps.tile([128, 512], f32, tag="qkacc",
                                                name=f"qkacc{which}_{mt}_{ncu}")
                                for kt in range(8):
                                    w_sb = (wq_sb if which == "q" else wkv_sb)
                                    nc.tensor.matmul(
                                        acc[:, 0:nw],
                                        w_sb[:, kt, mt * 128:(mt + 1) * 128],
                                        xT[ncu][:, kt, 0:nw],
                                        start=(kt == 0), stop=(kt == 7),
                                    )
                                for half in range(2):
                                    s = 2 * mt + half
                                    ncopy += 1
                                    eng = (nc.scalar.copy if ncopy % 2 else
                                           nc.vector.tensor_copy)
                                    eng(out=dst[0:64, s,
                                                ncu * 512:ncu * 512 + nw],
                                        in_=acc[64 * half:64 * half + 64, 0:nw])
                        for tt in range(4 * ncu, 4 * ncu + 4):
                            s0 = _smin(tt)
                            if s0 >= 4:
                                continue
                            nw = (4 - s0) * 64
                            acc = vps.tile([128, 256], f32, tag="vacc", name=f"vacc{tt}")
                            for kt in range(8):
                                nc.tensor.matmul(
                                    acc[:, 0:nw],
                                    xT[ncu][:, kt, (tt % 4) * 128:(tt % 4 + 1) * 128],
                                    wkv_sb[:, kt, 256 + s0 * 64:512],
                                    start=(kt == 0), stop=(kt == 7),
                                )
                            nc.vector.tensor_copy(
                                out=vext[tt][:, s0:4, 64:128],
                                in_=acc[:, 0:nw].rearrange("p (s d) -> p s d", d=64))

            # ---- phase 2: attention + output projection, per q-chunk ----
            # PSUM: stps 2x2-bank + shared ot/pacc pool 4x1-bank = 8 banks.
            with tc.tile_pool(name="stps", bufs=2, space="PSUM") as stps, \
                 tc.tile_pool(name="spool", bufs=4, space="PSUM") as spool:
                nosb = 0
                for qc in range(NQC):
                    on_tiles = [onp.tile([128, 512], f16, tag="on", name=f"on_{qc}_{p}")
                                for p in range(2)]
                    # big pair (slots 2,3) first: its chains gate the proj
                    for pair in (1, 0):
                        sA, sB = 2 * pair, 2 * pair + 1
                        nktA = min(SNKT[sA], 4 * qc + 4)
                        nktB = min(SNKT[sB], 4 * qc + 4)
                        # units: ("AB", kt) = slots A+B same kt in one 2-bank
                        # st tile; ("BB", kt) = slot B tiles kt, kt+1 (both
                        # full); ("B", kt) = slot B solo.
                        units = []
                        for kt in range(nktA):
                            units.append(("AB", kt))
                        rem = list(range(nktA, nktB))
                        i = 0
                        while i < len(rem):
                            kt = rem[i]
                            if (kt < 4 * qc and i + 1 < len(rem)
                                    and rem[i + 1] < 4 * qc):
                                units.append(("BB", kt))
                                i += 2
                            else:
                                units.append(("B", kt))
                                i += 1
                        ot = {s: spool.tile([128, 512], f32, tag="ot",
                                            name=f"ot_{qc}_{s}")
                              for s in (sA, sB)}
                        started = {sA: False, sB: False}
                        # (s, kt) list per unit for S-mm/mask/OT emission
                        def unit_tiles(u):
                            kind, kt = u
                            if kind == "AB":
                                return [(sA, kt, 0), (sB, kt, 1)]
                            if kind == "BB":
                                return [(sB, kt, 0), (sB, kt + 1, 1)]
                            return [(sB, kt, 0)]
                        n_ot = {sA: nktA, sB: nktB}
                        done_ot = {sA: 0, sB: 0}
                        with nc.named_scope(f"attn_q{qc}_p{pair}"):
                            for u in units:
                                st2 = stps.tile([128, 2, 512], f32, tag="st",
                                                name=f"st_{qc}_{pair}_{u[1]}")
                                p2 = ptp.tile([128, 2, 512], f16, tag="pt",
                                              name=f"pt_{qc}_{pair}_{u[1]}")
                                tl = unit_tiles(u)
                                j0s = [(kt - 4 * qc) * 128 if kt >= 4 * qc else 0
                                       for (_, kt, _) in tl]
                                for (s, kt, h), j0 in zip(tl, j0s):
                                    nc.tensor.matmul(
                                        st2[:, h, j0:512],
                                        kts[0:65, s, kt * 128:(kt + 1) * 128],
                                        qts[0:65, s, qc * 512 + j0:(qc + 1) * 512],
                                        start=True, stop=True,
                                    )
                                # one exp for the whole unit (no bias needed)
                                if u[0] == "AB" or u[0] == "BB":
                                    j0 = j0s[0]
                                    nc.scalar.activation(
                                        out=p2[:, :, j0:512], in_=st2[:, :, j0:512],
                                        func=EXP, bias=0.0, scale=0.125)
                                else:
                                    j0 = j0s[0]
                                    nc.scalar.activation(
                                        out=p2[:, 0, j0:512], in_=st2[:, 0, j0:512],
                                        func=EXP, bias=0.0, scale=0.125)
                                for (s, kt, h), j0 in zip(tl, j0s):
                                    diag = kt >= 4 * qc
                                    if diag:
                                        nc.gpsimd.tensor_mul(
                                            out=p2[:, h, j0:j0 + 128],
                                            in0=p2[:, h, j0:j0 + 128],
                                            in1=mask_sb,
                                        )
                                # OT: mask-free columns first (no mask wait),
                                # then the masked 128-block.
                                for (s, kt, h), j0 in zip(tl, j0s):
                                    diag = kt >= 4 * qc
                                    done_ot[s] += 1
                                    last = done_ot[s] == n_ot[s]
                                    segs = []
                                    if diag:
                                        if j0 + 128 < 512:
                                            segs.append((j0 + 128, 512, False))
                                        segs.append((j0, j0 + 128, True))
                                    else:
                                        segs.append((0, 512, False))
                                    for si, (a, b, is_blk) in enumerate(segs):
                                        nc.tensor.matmul(
                                            ot[s][:, a:b],
                                            vext[kt][:, s, :],
                                            p2[:, h, a:b],
                                            start=not started[s],
                                            stop=last and si == len(segs) - 1,
                                        )
                                        started[s] = True
                            # partitions 0:64 of ot hold the softmax
                            # denominator (base partition 0): fast-reciprocal
                            # straight from PSUM, then one DVE mul.
                            for s in (sA, sB):
                                r0 = 64 * (s % 2)
                                rec = rp.tile([64, 512], f32, tag="rec",
                                              name=f"rec_{qc}_{s}")
                                nc.vector.reciprocal_approx_fast(
                                    out=rec, in_=ot[s][0:64, :])
                                nc.vector.tensor_mul(
                                    out=on_tiles[pair][r0:r0 + 64, :],
                                    in0=ot[s][64:128, :],
                                    in1=rec,
                                )
                    with nc.named_scope(f"proj_q{qc}"):
                        for tloc in range(4):
                            tt = qc * 4 + tloc
                            osb = osp.tile([128, 1024], f16, tag="osb",
                                           name=f"osb_{tt}")
                            for ech in range(2):
                                pacc = spool.tile([128, 512], f32, tag="ot",
                                                  name=f"pacc_{tt}_{ech}")
                                for pt_i in (0, 1):
                                    nc.tensor.matmul(
                                        pacc,
                                        on_tiles[pt_i][:, tloc * 128:(tloc + 1) * 128],
                                        wp_sb[:, pt_i, ech * 512:(ech + 1) * 512],
                                        start=(pt_i == 0), stop=(pt_i == 1),
                                    )
                                nosb += 1
                                if nosb % 4 == 0:
                                    nc.scalar.copy(
                                        out=osb[:, ech * 512:(ech + 1) * 512],
                                        in_=pacc)
                                else:
                                    nc.vector.tensor_copy(
                                        out=osb[:, ech * 512:(ech + 1) * 512],
                                        in_=pacc)
                            nc.sync.dma_start(
                                out=out_d[tt * 128:(tt + 1) * 128, :], in_=osb)

    nc.finalize()
    _NC_CACHE["nc"] = nc
    return nc


def _prep_core_inputs(x, Wq, Aq, Bq, Wk, Ak, Bk, Wv, Av, Bv, Wp):
    """Host-side prep: LoRA fold, transposes, per-core slices."""
    slopes = _slopes()
    wq_m = Wq.astype(np.float64) + LORA_S * (Aq.astype(np.float64) @ Bq.astype(np.float64))
    wk_m = Wk.astype(np.float64) + LORA_S * (Ak.astype(np.float64) @ Bk.astype(np.float64))
    wv_m = Wv.astype(np.float64) + LORA_S * (Av.astype(np.float64) @ Bv.astype(np.float64))

    # shared [128,128] triangular mask: within a diagonal 128-block,
    # key-in-tile p is valid for local col j iff p <= j.
    p_i = np.arange(128)[:, None]
    j_i = np.arange(128)[None, :]
    masks = np.ascontiguousarray((p_i <= j_i).astype(np.float16))
    qones = np.ones((1, 4, 2048), dtype=np.float16)

    in_maps = []
    for c in range(8):
        b, g = divmod(c, 4)
        heads = [g, 4 + g, 8 + g, 12 + g]
        rows = np.concatenate([np.arange(h * DH, (h + 1) * DH) for h in heads])
        xT = x[b].T.astype(np.float16)          # [E, T]
        wqT = wq_m[rows, :].T.astype(np.float16)         # [E, 256]
        wkvT = np.concatenate(
            [wk_m[rows, :].T, wv_m[rows, :].T], axis=1).astype(np.float16)  # [E,512]
        wpT = Wp[:, rows].T.astype(np.float16)           # [256, E]
        kbias = np.zeros((1, 4, 2048), dtype=np.float16)
        for s, h in enumerate(heads):
            kbias[0, s, :] = (-8.0 * slopes[h] * np.arange(2048)).astype(np.float16)
        im = {
            "wqT": np.ascontiguousarray(
                wqT.reshape(8, 128, 256).transpose(1, 0, 2)),
            "wkvT": np.ascontiguousarray(
                wkvT.reshape(8, 128, 512).transpose(1, 0, 2)),
            "wpT": np.ascontiguousarray(
                wpT.reshape(2, 128, 1024).transpose(1, 0, 2)),
            "kbias": kbias, "qones": qones, "masks": masks,
        }
        for cch in range(NQC):
            im[f"xc{cch}"] = np.ascontiguousarray(
                xT[:, cch * 512:(cch + 1) * 512]
                .reshape(8, 128, 512).transpose(1, 0, 2))
        in_maps.append(im)
    return in_maps


def _run(in_maps, trace=False, **kw):
    from concourse.bass_utils import run_bass_kernel_spmd
    nc = _build_nc()
    return run_bass_kernel_spmd(nc, in_maps, core_ids=list(range(8)), trace=trace, **kw)


def kernel(x, Wq, Aq, Bq, Wk, Ak, Bk, Wv, Av, Bv, Wp):
    in_maps = _prep_core_inputs(x, Wq, Aq, Bq, Wk, Ak, Bk, Wv, Av, Bv, Wp)
    res = _run(in_maps)
    out = np.zeros((BATCH, T, E), dtype=np.float32)
    for c in range(8):
        out[c // 4] += res.results[c]["outp"].astype(np.float32)
    return out


# revision 7
# speedup vs baseline: 1.1203x; 1.0248x over previous
"""Trainium2 Bass kernel for nn_BaselineAttn (LoRA QKV + ALiBi causal attention).

Sharding: 8 cores SPMD, no collectives. Core c = (b, g): batch b = c // 4,
head group g = c % 4 handling heads [g, 4+g, 8+g, 12+g].

Host prep: LoRA folded into weights (W' = W + 2 A@B); x and weights
pre-transposed/sliced per core; partial outputs summed on host.

Device design (fp16 operands, fp32 PSUM):
  - feature-major x^T on chip -> q^T, k^T feature-major and v token-major
    from the same x^T; zero on-chip transposes.
  - attention in the S^T (key-major) orientation, with the ALiBi bias
    folded into the S matmul itself: k^T tiles carry a 65th contraction row
    holding -8*slope_h*k and q^T tiles carry a matching ones row, so
    S^T = k_ext^T.T @ q_ext^T already includes the softmax-shifted ALiBi
    term and the ScalarE exp needs no per-partition bias. That lets one
    activation cover BOTH slots of a pair (their S tiles live in one
    2-bank PSUM tile), halving the exp op count.
      P^T = exp(S^T/8), causal: only the 128-wide diagonal block of each
      diagonal-band tile is partially masked -> one shared [128,128]
      triangular mask multiply (GpSimd) per diagonal tile; the OT matmul of
      a diagonal tile is split into mask-free columns (issued right after
      exp) + the masked 128-block, hiding the mask latency.
      O^T += vext.T @ P^T where vext = [ones*64 | v]: the ones block
      replicates the softmax denominator onto partitions 0..63 so the DVE
      fast-reciprocal reads it straight from PSUM (base partition 0);
      normalize is recip + one DVE mul.
      out-partial = O^T_norm.T @ Wp'^T-slice, written f16 (host sums f32).
  - ALiBi gives key k weight exp(-slope_h*k); p is stored f16 whose
    subnormal floor is ~e^-17, so tiles beyond slope_h*128*kt > ~16 are
    exactly zero anyway and are skipped: SNKT = [1, 2, 8, 16].
  - inputs arrive via a few large contiguous DMAs (each dma_start costs
    ~600ns of HWDGE sequencer issue time), first-matmul working set first,
    split across the sync/scalar HWDGE rings; a short PE warm-up chain and
    a dummy exp (ACT table load) run inside the DMA window.
"""

import math

import numpy as np

E = 1024
H = 16
DH = 64
T = 2048
BATCH = 2
LORA_S = 2.0
NKT = T // 128          # 16 key tiles of 128
SNKT = [1, 2, 8, 16]    # per-slot key-tile caps (max over cores per slot)
NQC = 4                 # q chunks of 512
N_WARMUP = 8            # warm-up matmuls (N=256) to pre-warm the PE clock

_NC_CACHE = {}


def _slopes():
    start = 2 ** (-2 ** (-(math.log2(H) - 3)))
    return np.array([start * start**i for i in range(H)], dtype=np.float64)


def _smin(tt):
    """Lowest slot that still needs key-tile tt."""
    for s in range(4):
        if tt < SNKT[s]:
            return s
    return 4


def _build_nc():
    """Build the single SPMD Bass program (shared by all 8 cores)."""
    if "nc" in _NC_CACHE:
        return _NC_CACHE["nc"]

    from concourse.bacc import Bacc
    import concourse.tile as tile
    from concourse import mybir

    f16 = mybir.dt.float16
    f32 = mybir.dt.float32
    EXP = mybir.ActivationFunctionType.Exp

    nc = Bacc()

    xc_d = [nc.dram_tensor(f"xc{c}", [128, 8, 512], f16, kind="ExternalInput")
            for c in range(NQC)]
    wq_d = nc.dram_tensor("wqT", [128, 8, 256], f16, kind="ExternalInput")
    wkv_d = nc.dram_tensor("wkvT", [128, 8, 512], f16, kind="ExternalInput")
    wp_d = nc.dram_tensor("wpT", [128, 2, 1024], f16, kind="ExternalInput")
    kbias_d = nc.dram_tensor("kbias", [1, 4, 2048], f16, kind="ExternalInput")
    qones_d = nc.dram_tensor("qones", [1, 4, 2048], f16, kind="ExternalInput")
    mask_d = nc.dram_tensor("masks", [128, 128], f16, kind="ExternalInput")
    out_d = nc.dram_tensor("outp", [T, E], f16, kind="ExternalOutput")
    scr_d = nc.dram_tensor("scratch", [128, 8], f16, kind="ExternalOutput")

    with tile.TileContext(nc) as tc:
        with (
            tc.tile_pool(name="persist", bufs=1) as pp,
            tc.tile_pool(name="ptpool", bufs=8) as ptp,
            tc.tile_pool(name="onorm", bufs=4) as onp,
            tc.tile_pool(name="rpool", bufs=4) as rp,
            tc.tile_pool(name="outsb", bufs=6) as osp,
        ):
            # ---- PE warm-up source, independent of DMA ----
            wm_sb = pp.tile([128, 256], f16, name="wm_sb")
            nc.gpsimd.memset(wm_sb, 0.5)
            wexp = pp.tile([128, 8], f16, name="wexp")
            scr_sb = pp.tile([128, 8], f16, name="scr_sb")

            # ---- input loads: few, large, contiguous; first-MM set first --
            xT = [pp.tile([128, 8, 512], f16, name=f"xT{c}") for c in range(NQC)]
            wq_sb = pp.tile([128, 8, 256], f16, name="wq_sb")
            wkv_sb = pp.tile([128, 8, 512], f16, name="wkv_sb")
            # q_ext/k_ext: [65, slot, keys]: rows 0:64 features, row 64 =
            # ones (q) / -8*slope*k ALiBi bias row (k).
            qts = pp.tile([65, 4, 2048], f16, name="qts")
            kts = pp.tile([65, 4, 2048], f16, name="kts")

            # x chunks feed the PE critical path: give them the whole sync
            # ring; weights ride the scalar ring; tiny loads go SWDGE (Pool).
            nc.sync.dma_start(out=xT[0], in_=xc_d[0][:, :, :])
            nc.scalar.dma_start(out=wq_sb, in_=wq_d[:, :, :])
            for c in range(1, NQC):
                nc.sync.dma_start(out=xT[c], in_=xc_d[c][:, :, :])
            nc.scalar.dma_start(out=wkv_sb, in_=wkv_d[:, :, :])
            nc.gpsimd.dma_start(out=kts[64:65, :, :], in_=kbias_d[:, :, :])
            nc.gpsimd.dma_start(out=qts[64:65, :, :], in_=qones_d[:, :, :])
            mask_sb = pp.tile([128, 128], f16, name="mask")
            nc.gpsimd.dma_start(out=mask_sb, in_=mask_d[:, :])
            wp_sb = pp.tile([128, 2, 1024], f16, name="wp_sb")
            nc.scalar.dma_start(out=wp_sb, in_=wp_d[:, :, :])
            # dummy exp hoists the ~1.3us ACT table load into the DMA window
            # (after the scalar-ring DMA triggers so it doesn't delay them).
            nc.scalar.activation(out=wexp, in_=wm_sb[:, 0:8], func=EXP,
                                 bias=0.0, scale=0.125)

            # warm-up matmuls cover the PE-idle DMA window so the HAM clock
            # gate is released before real work; results are dead.
            with tc.tile_pool(name="wups", bufs=1, space="PSUM") as wups:
                wacc = wups.tile([128, 512], f32, name="wacc")
                for _ in range(N_WARMUP):
                    nc.tensor.matmul(wacc[:, 0:256], wm_sb[:, 0:128], wm_sb,
                                     start=True, stop=True)
                # tiny live sink so the chain can't be dead-code-eliminated
                nc.vector.tensor_copy(out=scr_sb, in_=wacc[:, 0:8])
                nc.sync.dma_start(out=scr_d[:, :], in_=scr_sb)

            # vext[tt]: [128 keys, slot, 128]: cols 0:64 = ones (denominator
            # replicas at base partition 0), cols 64:128 = v.
            vext = []
            for tt in range(NKT):
                v_t = pp.tile([128, 4, 128], f16, name=f"vext{tt}")
                nc.gpsimd.memset(v_t[:, :, 0:64], 1.0)
                vext.append(v_t)

            ncopy = 0  # round-robin Act/DVE for qk PSUM->SBUF copies

            # ---- phase 1: QKV projections ----
            with tc.tile_pool(name="qkps", bufs=3, space="PSUM") as qkps, \
                 tc.tile_pool(name="vps", bufs=3, space="PSUM") as vps:
                # chunk-major emission: q,k,v for chunk ncu before chunk ncu+1,
                # so attention for q-chunk 0 can start 4x earlier.
                with nc.named_scope("qkv_proj"):
                    for ncu in range(NQC):
                        for which, dst in (("q", qts), ("k", kts)):
                            for mt in range(2):
                                if which == "k" and mt == 0 and ncu >= 1:
                                    continue  # slots 0,1 need keys < 256 only
                                nw = 256 if (which == "k" and mt == 0) else 512
                                acc = qkps.tile([128, 512], f32, tag="qkacc",
                                                name=f"qkacc{which}_{mt}_{ncu}")
                                for kt in range(8):
                                    w_sb = (wq_sb if which == "q" else wkv_sb)
                                    nc.tensor.matmul(
                                        acc[:, 0:nw],
                                        w_sb[:, kt, mt * 128:(mt + 1) * 128],
                                        xT[ncu][:, kt, 0:nw],
                                        start=(kt == 0), stop=(kt == 7),
                                    )
                                for half in range(2):
                                    s = 2 * mt + half
                                    ncopy += 1
                                    eng = (nc.scalar.copy if ncopy % 2 else
                                           nc.vector.tensor_copy)
                                    eng(out=dst[0:64, s,
                                                ncu * 512:ncu * 512 + nw],
                                        in_=acc[64 * half:64 * half + 64, 0:nw])
                        for tt in range(4 * ncu, 4 * ncu + 4):
                            s0 = _smin(tt)
                            if s0 >= 4:
                                continue
                            nw = (4 - s0) * 64
                            acc = vps.tile([128, 256], f32, tag="vacc", name=f"vacc{tt}")
                            for kt in range(8):
                                nc.tensor.matmul(
                                    acc[:, 0:nw],
                                    xT[ncu][:, kt, (tt % 4) * 128:(tt % 4 + 1) * 128],
                                    wkv_sb[:, kt, 256 + s0 * 64:512],
                                    start=(kt == 0), stop=(kt == 7),
                                )
                            nc.vector.tensor_copy(
                                out=vext[tt][:, s0:4, 64:128],
                                in_=acc[:, 0:nw].rearrange("p (s d) -> p s d", d=64))

            # ---- phase 2: attention + output projection, per q-chunk ----
            # PSUM: stps 2x2-bank + shared ot/pacc pool 4x1-bank = 8 banks.
            with tc.tile_pool(name="stps", bufs=2, space="PSUM") as stps, \
                 tc.tile_pool(name="spool", bufs=4, space="PSUM") as spool:
                nosb = 0
                for qc in range(NQC):
                    on_tiles = [onp.tile([128, 512], f16, tag="on", name=f"on_{qc}_{p}")
                                for p in range(2)]
                    # big pair (slots 2,3) first: its chains gate the proj
                    for pair in (1, 0):
                        sA, sB = 2 * pair, 2 * pair + 1
                        nktA = min(SNKT[sA], 4 * qc + 4)
                        nktB = min(SNKT[sB], 4 * qc + 4)
                        # units: ("AB", kt) = slots A+B same kt in one 2-bank
                        # st tile; ("BB", kt) = slot B tiles kt, kt+1 (both
                        # full); ("B", kt) = slot B solo.
                        units = []
                        for kt in range(nktA):
                            units.append(("AB", kt))
                        rem = list(range(nktA, nktB))
                        i = 0
                        while i < len(rem):
                            kt = rem[i]
                            if (kt < 4 * qc and i + 1 < len(rem)
                                    and rem[i + 1] < 4 * qc):
                                units.append(("BB", kt))
                                i += 2
                            else:
                                units.append(("B", kt))
                                i += 1
                        ot = {s: spool.tile([128, 512], f32, tag="ot",
                                            name=f"ot_{qc}_{s}")
                              for s in (sA, sB)}
                        started = {sA: False, sB: False}
                        # (s, kt) list per unit for S-mm/mask/OT emission
                        def unit_tiles(u):
                            kind, kt = u
                            if kind == "AB":
                                return [(sA, kt, 0), (sB, kt, 1)]
                            if kind == "BB":
                                return [(sB, kt, 0), (sB, kt + 1, 1)]
                            return [(sB, kt, 0)]
                        n_ot = {sA: nktA, sB: nktB}
                        done_ot = {sA: 0, sB: 0}
                        with nc.named_scope(f"attn_q{qc}_p{pair}"):
                            for u in units:
                                st2 = stps.tile([128, 2, 512], f32, tag="st",
                                                name=f"st_{qc}_{pair}_{u[1]}")
                                p2 = ptp.tile([128, 2, 512], f16, tag="pt",
                                              name=f"pt_{qc}_{pair}_{u[1]}")
                                tl = unit_tiles(u)
                                j0s = [(kt - 4 * qc) * 128 if kt >= 4 * qc else 0
                                       for (_, kt, _) in tl]
                                for (s, kt, h), j0 in zip(tl, j0s):
                                    nc.tensor.matmul(
                                        st2[:, h, j0:512],
                                        kts[0:65, s, kt * 128:(kt + 1) * 128],
                                        qts[0:65, s, qc * 512 + j0:(qc + 1) * 512],
                                        start=True, stop=True,
                                    )
                                # one exp for the whole unit (no bias needed)
                                if u[0] == "AB" or u[0] == "BB":
                                    j0 = j0s[0]
                                    nc.scalar.activation(
                                        out=p2[:, :, j0:512], in_=st2[:, :, j0:512],
                                        func=EXP, bias=0.0, scale=0.125)
                                else:
                                    j0 = j0s[0]
                                    nc.scalar.activation(
                                        out=p2[:, 0, j0:512], in_=st2[:, 0, j0:512],
                                        func=EXP, bias=0.0, scale=0.125)
                                for (s, kt, h), j0 in zip(tl, j0s):
                                    diag = kt >= 4 * qc
                                    if diag:
                                        nc.gpsimd.tensor_mul(
                                            out=p2[:, h, j0:j0 + 128],
                                            in0=p2[:, h, j0:j0 + 128],
                                            in1=mask_sb,
                                        )
                                # OT: mask-free columns first (no mask wait),
                                # then the masked 128-block.
                                for (s, kt, h), j0 in zip(tl, j0s):
                                    diag = kt >= 4 * qc
                                    done_ot[s] += 1
                                    last = done_ot[s] == n_ot[s]
                                    segs = []
                                    if diag:
                                        if j0 + 128 < 512:
                                            segs.append((j0 + 128, 512, False))
                                        segs.append((j0, j0 + 128, True))
                                    else:
                                        segs.append((0, 512, False))
                                    for si, (a, b, is_blk) in enumerate(segs):
                                        nc.tensor.matmul(
                                            ot[s][:, a:b],
                                            vext[kt][:, s, :],
                                            p2[:, h, a:b],
                                            start=not started[s],
                                            stop=last and si == len(segs) - 1,
                                        )
                                        started[s] = True
                            # partitions 0:64 of ot hold the softmax
                            # denominator (base partition 0): fast-reciprocal
                            # straight from PSUM, then one DVE mul.
                            for s in (sA, sB):
                                r0 = 64 * (s % 2)
                                rec = rp.tile([64, 512], f32, tag="rec",
                                              name=f"rec_{qc}_{s}")
                                nc.vector.reciprocal_approx_fast(
                                    out=rec, in_=ot[s][0:64, :])
                                nc.vector.tensor_mul(
                                    out=on_tiles[pair][r0:r0 + 64, :],
                                    in0=ot[s][64:128, :],
                                    in1=rec,
                                )
                    with nc.named_scope(f"proj_q{qc}"):
                        for tloc in range(4):
                            tt = qc * 4 + tloc
                            osb = osp.tile([128, 1024], f16, tag="osb",
                                           name=f"osb_{tt}")
                            for ech in range(2):
                                pacc = spool.tile([128, 512], f32, tag="ot",
                                                  name=f"pacc_{tt}_{ech}")
                                for pt_i in (0, 1):
                                    nc.tensor.matmul(
                                        pacc,
                                        on_tiles[pt_i][:, tloc * 128:(tloc + 1) * 128],
                                        wp_sb[:, pt_i, ech * 512:(ech + 1) * 512],
                                        start=(pt_i == 0), stop=(pt_i == 1),
                                    )
                                nosb += 1
                                if nosb % 4 == 0:
                                    nc.scalar.copy(
                                        out=osb[:, ech * 512:(ech + 1) * 512],
                                        in_=pacc)
                                else:
                                    nc.vector.tensor_copy(
                                        out=osb[:, ech * 512:(ech + 1) * 512],
                                        in_=pacc)
                            nc.sync.dma_start(
                                out=out_d[tt * 128:(tt + 1) * 128, :], in_=osb)

    nc.finalize()
    _NC_CACHE["nc"] = nc
    return nc


def _prep_core_inputs(x, Wq, Aq, Bq, Wk, Ak, Bk, Wv, Av, Bv, Wp):
    """Host-side prep: LoRA fold, transposes, per-core slices."""
    slopes = _slopes()
    wq_m = Wq.astype(np.float64) + LORA_S * (Aq.astype(np.float64) @ Bq.astype(np.float64))
    wk_m = Wk.astype(np.float64) + LORA_S * (Ak.astype(np.float64) @ Bk.astype(np.float64))
    wv_m = Wv.astype(np.float64) + LORA_S * (Av.astype(np.float64) @ Bv.astype(np.float64))

    # shared [128,128] triangular mask: within a diagonal 128-block,
    # key-in-tile p is valid for local col j iff p <= j.
    p_i = np.arange(128)[:, None]
    j_i = np.arange(128)[None, :]
    masks = np.ascontiguousarray((p_i <= j_i).astype(np.float16))
    qones = np.ones((1, 4, 2048), dtype=np.float16)

    in_maps = []
    for c in range(8):
        b, g = divmod(c, 4)
        heads = [g, 4 + g, 8 + g, 12 + g]
        rows = np.concatenate([np.arange(h * DH, (h + 1) * DH) for h in heads])
        xT = x[b].T.astype(np.float16)          # [E, T]
        wqT = wq_m[rows, :].T.astype(np.float16)         # [E, 256]
        wkvT = np.concatenate(
            [wk_m[rows, :].T, wv_m[rows, :].T], axis=1).astype(np.float16)  # [E,512]
        wpT = Wp[:, rows].T.astype(np.float16)           # [256, E]
        kbias = np.zeros((1, 4, 2048), dtype=np.float16)
        for s, h in enumerate(heads):
            kbias[0, s, :] = (-8.0 * slopes[h] * np.arange(2048)).astype(np.float16)
        im = {
            "wqT": np.ascontiguousarray(
                wqT.reshape(8, 128, 256).transpose(1, 0, 2)),
            "wkvT": np.ascontiguousarray(
                wkvT.reshape(8, 128, 512).transpose(1, 0, 2)),
            "wpT": np.ascontiguousarray(
                wpT.reshape(2, 128, 1024).transpose(1, 0, 2)),
            "kbias": kbias, "qones": qones, "masks": masks,
        }
        for cch in range(NQC):
            im[f"xc{cch}"] = np.ascontiguousarray(
                xT[:, cch * 512:(cch + 1) * 512]
                .reshape(8, 128, 512).transpose(1, 0, 2))
        in_maps.append(im)
    return in_maps


def _run(in_maps, trace=False, **kw):
    from concourse.bass_utils import run_bass_kernel_spmd
    nc = _build_nc()
    return run_bass_kernel_spmd(nc, in_maps, core_ids=list(range(8)), trace=trace, **kw)


def kernel(x, Wq, Aq, Bq, Wk, Ak, Bk, Wv, Av, Bv, Wp):
    in_maps = _prep_core_inputs(x, Wq, Aq, Bq, Wk, Ak, Bk, Wv, Av, Bv, Wp)
    res = _run(in_maps)
    out = np.zeros((BATCH, T, E), dtype=np.float32)
    for c in range(8):
        out[c // 4] += res.results[c]["outp"].astype(np.float32)
    return out
